# revision 5
# baseline (speedup 1.0000x reference)
"""Trainium2 Bass kernel for a 4-layer transformer (B=2,S=1024,D=1024,H=16,F=4096,V=32000).

Strategy (8 NeuronCores):
 - Sequence-parallel transformer layers: each core owns 256 tokens
   (cores 0-3: batch 0, cores 4-7: batch 1). All layer weights replicated
   (streamed from HBM as bf16). Activations kept feature-major
   ([d on partitions, tokens on free]) so no transposes are needed in the
   layer loop; per-token stats (LN mean/var, softmax 1/Z) are computed with
   ones-matmuls and broadcast back with K=1 matmuls.
 - Attention: per-layer AllGather of K/V (bf16) within each batch's 4-core
   group; scores computed transposed (s^T[kt,q]) so softmax-weighted sums
   contract on the partition axis without transposing P.
 - LM head: final LN output AllGather-ed across all 8 cores; each core
   computes a 4000-wide vocab shard of the logits for all 2048 tokens.

Self-contained: hardcodes all shapes; host side only reshapes/shards/casts.
"""
import numpy as np
import ml_dtypes

import concourse.bass as bass
import concourse.bacc as bacc
import concourse.mybir as mybir
import concourse.tile as tile
from concourse import bass_utils
from concourse.masks import make_identity

B, S, D, H, L, F, V = 2, 1024, 1024, 16, 4, 4096, 32000
DH = D // H          # 64
NCORES = 8
T = (B * S) // NCORES  # 256 tokens per core
NT = B * S             # 2048
VS = V // NCORES       # 4000
VSP = 4096             # padded vocab shard
P = 128
ND = D // P            # 8 d-tiles
NFT = F // P           # 32 fc1 f-tiles

f32 = mybir.dt.float32
bf16 = mybir.dt.bfloat16
i32 = mybir.dt.int32
AF = mybir.ActivationFunctionType
OP = mybir.AluOpType


def _ln(nc, ps, act, rows, cons, x_sb, s_t, b_t, out_h, ident_unused=None):
    """LayerNorm: x_sb [128, 8, 256] f32 -> out_h [128, 8, 256] bf16.

    s_t/b_t: [128, 8] f32 param tiles (column dt = scale/bias for features
    dt*128..dt*128+127)."""
    stat_x = ps.tile([1, 256], f32, tag="aux")
    stat_sq = ps.tile([1, 256], f32, tag="aux")
    for dt in range(ND):
        sq = act.tile([P, 256], f32, tag="sq")
        nc.vector.tensor_mul(sq[:], x_sb[:, dt, :], x_sb[:, dt, :])
        nc.tensor.matmul(stat_x[:], lhsT=cons.ones_col_f32[:], rhs=x_sb[:, dt, :],
                         start=(dt == 0), stop=(dt == ND - 1))
        nc.tensor.matmul(stat_sq[:], lhsT=cons.ones_col_f32[:], rhs=sq[:],
                         start=(dt == 0), stop=(dt == ND - 1))
    murow = rows.tile([1, 512], f32, tag="row")  # [mu | invstd]
    nc.scalar.activation(murow[:, 0:256], stat_x[:], AF.Copy, scale=1.0 / D)
    msq = rows.tile([1, 256], f32, tag="row")
    nc.scalar.activation(msq[:], stat_sq[:], AF.Copy, scale=1.0 / D)
    var = rows.tile([1, 256], f32, tag="row")
    nc.vector.tensor_mul(var[:], murow[:, 0:256], murow[:, 0:256])
    nc.vector.tensor_sub(var[:], msq[:], var[:])
    std = rows.tile([1, 256], f32, tag="row")
    nc.scalar.activation(std[:], var[:], AF.Sqrt, bias=cons.eps_row[:, 0:1])
    nc.vector.reciprocal(murow[:, 256:512], std[:])
    bc = ps.tile([P, 512], f32, tag="acc")
    nc.tensor.matmul(bc[:], lhsT=cons.ones_row_f32[:], rhs=murow[:],
                     start=True, stop=True)
    for dt in range(ND):
        t = act.tile([P, 256], f32, tag="sq")
        nc.vector.tensor_sub(t[:], x_sb[:, dt, :], bc[:, 0:256])
        nc.vector.tensor_mul(t[:], t[:], bc[:, 256:512])
        nc.vector.tensor_scalar(out_h[:, dt, :], t[:], s_t[:, dt:dt + 1],
                                b_t[:, dt:dt + 1], OP.mult, OP.add)


class _Cons:
    pass


def build(n_layers=L):
    nc = bacc.Bacc("TRN2", target_bir_lowering=False, debug=False,
                   num_devices=NCORES)

    ids = nc.dram_tensor("ids", [T], i32, kind="ExternalInput").ap()
    pos = nc.dram_tensor("pos", [T, D], f32, kind="ExternalInput").ap()
    embed_w = nc.dram_tensor("embed_w", [V, D], f32, kind="ExternalInput").ap()
    attn_wT = nc.dram_tensor("attn_wT", [L, D, 3 * D], bf16, kind="ExternalInput").ap()
    attn_in_b = nc.dram_tensor("attn_in_b", [L, 3 * D], f32, kind="ExternalInput").ap()
    proj_wT = nc.dram_tensor("proj_wT", [L, D, D], bf16, kind="ExternalInput").ap()
    proj_b = nc.dram_tensor("proj_b", [L, D], f32, kind="ExternalInput").ap()
    fc1_wT = nc.dram_tensor("fc1_wT", [L, D, F], bf16, kind="ExternalInput").ap()
    fc1_b = nc.dram_tensor("fc1_b", [L, F], f32, kind="ExternalInput").ap()
    fc2_wT = nc.dram_tensor("fc2_wT", [L, F, D], bf16, kind="ExternalInput").ap()
    fc2_b = nc.dram_tensor("fc2_b", [L, D], f32, kind="ExternalInput").ap()
    ln1_s = nc.dram_tensor("ln1_s", [L, D], f32, kind="ExternalInput").ap()
    ln1_b = nc.dram_tensor("ln1_b", [L, D], f32, kind="ExternalInput").ap()
    ln2_s = nc.dram_tensor("ln2_s", [L, D], f32, kind="ExternalInput").ap()
    ln2_b = nc.dram_tensor("ln2_b", [L, D], f32, kind="ExternalInput").ap()
    lnf_s = nc.dram_tensor("lnf_s", [D], f32, kind="ExternalInput").ap()
    lnf_b = nc.dram_tensor("lnf_b", [D], f32, kind="ExternalInput").ap()
    lm_wT = nc.dram_tensor("lm_wT", [D, VSP], bf16, kind="ExternalInput").ap()
    lm_b = nc.dram_tensor("lm_b", [VSP], f32, kind="ExternalInput").ap()
    outT = nc.dram_tensor("outT", [VSP, NT], f32, kind="ExternalOutput").ap()

    kv_groups = [[0, 1, 2, 3], [4, 5, 6, 7]]
    all_group = [list(range(NCORES))]

    with tile.TileContext(nc) as tc:
        with (
            tc.tile_pool(name="consp", bufs=1) as consp,
            tc.tile_pool(name="wpool", bufs=8) as wpool,
            tc.tile_pool(name="act", bufs=1) as act,
            tc.tile_pool(name="rows", bufs=6) as rows,
            tc.tile_pool(name="par", bufs=2) as par,
            tc.tile_pool(name="ps", bufs=2, space="PSUM") as ps,
            tc.tile_pool(name="dram", bufs=1, space="DRAM") as dram,
        ):
            cons = _Cons()
            ident = consp.tile([P, P], f32)
            make_identity(nc, ident)
            ones_col_f32 = consp.tile([P, 1], f32)
            nc.vector.memset(ones_col_f32[:], 1.0)
            ones_col_bf = consp.tile([P, 1], bf16)
            nc.vector.memset(ones_col_bf[:], 1.0)
            ones_row_f32 = consp.tile([1, P], f32)
            nc.vector.memset(ones_row_f32[:], 1.0)
            cons.ones_col_f32 = ones_col_f32
            cons.ones_row_f32 = ones_row_f32
            eps_row = consp.tile([1, 1], f32)
            nc.vector.memset(eps_row[:], 1e-5)
            cons.eps_row = eps_row

            x_sb = consp.tile([P, ND, 256], f32)  # residual, feature-major

            # ---------------- embedding ----------------
            for tc2 in range(2):
                ids_sb = par.tile([P, 1], i32, tag="ids")
                nc.sync.dma_start(ids_sb[:], ids[tc2 * P:(tc2 + 1) * P, None])
                gat = wpool.tile([P, D], f32, tag="w")
                nc.gpsimd.indirect_dma_start(
                    out=gat[:], out_offset=None, in_=embed_w[:],
                    in_offset=bass.IndirectOffsetOnAxis(ap=ids_sb[:, :1], axis=0))
                pos_sb = wpool.tile([P, D], f32, tag="w")
                nc.sync.dma_start(pos_sb[:], pos[tc2 * P:(tc2 + 1) * P, :])
                nc.vector.tensor_add(gat[:], gat[:], pos_sb[:])
                for dt in range(ND):
                    tp = ps.tile([P, P], f32, tag="att_s")
                    nc.tensor.transpose(tp[:], gat[:, dt * P:(dt + 1) * P], ident[:])
                    nc.vector.tensor_copy(x_sb[:, dt, tc2 * P:(tc2 + 1) * P], tp[:])

            # ---------------- layers ----------------
            for l in range(n_layers):
                # LN1
                ln1s_t = par.tile([P, ND], f32, tag="lnp")
                nc.sync.dma_start(ln1s_t[:], ln1_s[l].rearrange("(k p) -> p k", p=P))
                ln1b_t = par.tile([P, ND], f32, tag="lnp")
                nc.sync.dma_start(ln1b_t[:], ln1_b[l].rearrange("(k p) -> p k", p=P))
                h_sb = act.tile([P, ND, 256], bf16, tag="h", bufs=2)
                _ln(nc, ps, act, rows, cons, x_sb, ln1s_t, ln1b_t, h_sb)

                # QKV weights: 8 d-slices of [128, 3072]
                w_qkv = []
                for dt in range(ND):
                    wt = wpool.tile([P, 4096], bf16, tag="w", name=f"wqkv{l}_{dt}")
                    nc.sync.dma_start(wt[:, 0:3 * D], attn_wT[l, dt * P:(dt + 1) * P, :])
                    w_qkv.append(wt)
                qkvb_t = par.tile([P, 24], f32, tag="qkvb")
                nc.sync.dma_start(qkvb_t[:], attn_in_b[l].rearrange("(k p) -> p k", p=P))

                q_all = act.tile([P, 8, 256], bf16, tag="q")
                k_loc = act.tile([P, 8, 256], bf16, tag="kloc")
                for ft in range(16):
                    acc = ps.tile([P, 256], f32, tag="acc")
                    for dt in range(ND):
                        nc.tensor.matmul(acc[:], lhsT=w_qkv[dt][:, ft * P:(ft + 1) * P],
                                         rhs=h_sb[:, dt, :],
                                         start=(dt == 0), stop=(dt == ND - 1))
                    dst = q_all[:, ft, :] if ft < 8 else k_loc[:, ft - 8, :]
                    nc.scalar.activation(dst, acc[:], AF.Identity,
                                         bias=qkvb_t[:, ft:ft + 1])
                # V (token-major): out [128 tok, 1024 f]
                v_loc = act.tile([P, 2, D], bf16, tag="vloc")
                for tc2 in range(2):
                    for nb in range(2):
                        acc = ps.tile([P, 512], f32, tag="acc")
                        for dt in range(ND):
                            nc.tensor.matmul(
                                acc[:], lhsT=h_sb[:, dt, tc2 * P:(tc2 + 1) * P],
                                rhs=w_qkv[dt][:, 2 * D + nb * 512:2 * D + (nb + 1) * 512],
                                start=(dt == 0), stop=(dt == ND - 1))
                        nc.scalar.activation(v_loc[:, tc2, nb * 512:(nb + 1) * 512],
                                             acc[:], AF.Copy)

                # bounce k/v to DRAM and AllGather within the 4-core group
                kv_in = dram.tile([2 * 256 * D], bf16, tag="kvin", name=f"kvin{l}")
                kv_out = dram.tile([4, 2 * 256 * D], bf16, tag="kvout", name=f"kvout{l}")
                kv_in_k = kv_in.rearrange("(a f p t) -> a f p t", a=2, f=8, p=P, t=256)[0]
                kv_in_v = kv_in.rearrange("(a t f) -> a t f", a=2, t=256, f=D)[1]
                nc.sync.dma_start(kv_in_k.rearrange("f p t -> p f t"), k_loc[:])
                for tc2 in range(2):
                    nc.sync.dma_start(kv_in_v[tc2 * P:(tc2 + 1) * P, :],
                                      v_loc[:, tc2, :])
                nc.gpsimd.collective_compute(
                    "AllGather", OP.bypass, replica_groups=kv_groups,
                    ins=[kv_in.opt()], outs=[kv_out.opt()])
                kv_out_k = kv_out.rearrange("r (a f p t) -> r a f p t",
                                            a=2, f=8, p=P, t=256)[:, 0]
                kv_out_v = kv_out.rearrange("r (a t f) -> r a t f",
                                            a=2, t=256, f=D)[:, 1]

                # load gathered K (feature-major) and V' (token-major + ones col)
                k_sb = act.tile([P, 8, 1024], bf16, tag="ksb")
                for j in range(8):
                    nc.sync.dma_start(
                        k_sb[:, j, :].rearrange("p (r t) -> p r t", r=4),
                        kv_out_k[:, j, :, :].rearrange("r p t -> p r t"))
                v_sb8 = act.tile([P, 8, 16 * 65], bf16, tag="vsb8")
                for c in range(8):
                    r, th = c // 2, c % 2
                    dst = v_sb8[:, c, :].rearrange("p (h g) -> p h g", h=16, g=65)
                    nc.sync.dma_start(
                        dst[:, :, 0:64],
                        kv_out_v[r, th * P:(th + 1) * P, :].rearrange(
                            "t (h f) -> t h f", h=16))
                    nc.vector.memset(dst[:, :, 64:65], 1.0)

                # attention per head
                o_sb = act.tile([P, ND, 256], bf16, tag="o")
                scale = 1.0 / np.sqrt(DH)
                for j in range(8):
                    for hh in range(2):
                        h_idx = 2 * j + hh
                        base = hh * 64
                        av = ps.tile([P, 256], f32, tag="av")
                        z = ps.tile([1, 256], f32, tag="aux")
                        for c in range(8):
                            sps = ps.tile([P, 256], f32, tag="att_s")
                            nc.tensor.matmul(
                                sps[:], lhsT=k_sb[base:base + 64, j, c * P:(c + 1) * P],
                                rhs=q_all[base:base + 64, j, :], start=True, stop=True)
                            e = act.tile([P, 256], bf16, tag="e", bufs=3)
                            nc.scalar.activation(e[:], sps[:], AF.Exp, scale=scale)
                            nc.tensor.matmul(
                                av[base:base + 64, :],
                                lhsT=v_sb8[:, c, h_idx * 65:h_idx * 65 + 64],
                                rhs=e[:], start=(c == 0), stop=(c == 7),
                                tile_position=(0, base))
                            nc.tensor.matmul(z[:], lhsT=ones_col_bf[:], rhs=e[:],
                                             start=(c == 0), stop=(c == 7))
                        recip = rows.tile([1, 256], f32, tag="row")
                        nc.vector.reciprocal(recip[:], z[:])
                        bc = ps.tile([P, 256], f32, tag="aux")
                        nc.tensor.matmul(bc[base:base + 64, :],
                                         lhsT=ones_row_f32[:, 0:64], rhs=recip[:],
                                         start=True, stop=True,
                                         tile_position=(0, base))
                        bc_sb = act.tile([P, 256], f32, tag="bcsb", bufs=2)
                        nc.vector.tensor_copy(bc_sb[base:base + 64, :],
                                              bc[base:base + 64, :])
                        nc.vector.tensor_mul(o_sb[base:base + 64, j, :],
                                             av[base:base + 64, :],
                                             bc_sb[base:base + 64, :])
                    # + v bias (valid because sum of softmax weights == 1)
                    nc.vector.tensor_scalar_add(o_sb[:, j, :], o_sb[:, j, :],
                                                qkvb_t[:, 16 + j:16 + j + 1])

                # attention out-proj + residual
                w_proj = []
                for dt in range(ND):
                    wt = wpool.tile([P, 4096], bf16, tag="w", name=f"wproj{l}_{dt}")
                    nc.sync.dma_start(wt[:, 0:D], proj_wT[l, dt * P:(dt + 1) * P, :])
                    w_proj.append(wt)
                projb_t = par.tile([P, ND], f32, tag="lnp")
                nc.sync.dma_start(projb_t[:], proj_b[l].rearrange("(k p) -> p k", p=P))
                for do in range(ND):
                    acc = ps.tile([P, 256], f32, tag="acc")
                    for dt in range(ND):
                        nc.tensor.matmul(acc[:], lhsT=w_proj[dt][:, do * P:(do + 1) * P],
                                         rhs=o_sb[:, dt, :],
                                         start=(dt == 0), stop=(dt == ND - 1))
                    nc.vector.scalar_tensor_tensor(
                        out=x_sb[:, do, :], in0=acc[:], scalar=projb_t[:, do:do + 1],
                        in1=x_sb[:, do, :], op0=OP.add, op1=OP.add)

                # LN2 + MLP
                ln2s_t = par.tile([P, ND], f32, tag="lnp")
                nc.sync.dma_start(ln2s_t[:], ln2_s[l].rearrange("(k p) -> p k", p=P))
                ln2b_t = par.tile([P, ND], f32, tag="lnp")
                nc.sync.dma_start(ln2b_t[:], ln2_b[l].rearrange("(k p) -> p k", p=P))
                h2_sb = act.tile([P, ND, 256], bf16, tag="h", bufs=2)
                _ln(nc, ps, act, rows, cons, x_sb, ln2s_t, ln2b_t, h2_sb)

                w_fc1 = []
                for dt in range(ND):
                    wt = wpool.tile([P, 4096], bf16, tag="w", name=f"wfc1{l}_{dt}")
                    nc.sync.dma_start(wt[:], fc1_wT[l, dt * P:(dt + 1) * P, :])
                    w_fc1.append(wt)
                fc1b_t = par.tile([P, NFT], f32, tag="fcb")
                nc.sync.dma_start(fc1b_t[:], fc1_b[l].rearrange("(k p) -> p k", p=P))
                h1g = act.tile([P, NFT, 256], bf16, tag="h1g")
                for ft in range(NFT):
                    acc = ps.tile([P, 256], f32, tag="acc")
                    for dt in range(ND):
                        nc.tensor.matmul(acc[:], lhsT=w_fc1[dt][:, ft * P:(ft + 1) * P],
                                         rhs=h2_sb[:, dt, :],
                                         start=(dt == 0), stop=(dt == ND - 1))
                    nc.scalar.activation(h1g[:, ft, :], acc[:], AF.Gelu,
                                         bias=fc1b_t[:, ft:ft + 1])

                w_fc2 = []
                for g in range(ND):
                    wt = wpool.tile([P, 4, D], bf16, tag="w", name=f"wfc2{l}_{g}")
                    nc.sync.dma_start(
                        wt[:], fc2_wT[l, g * 512:(g + 1) * 512, :].rearrange(
                            "(i p) d -> p i d", p=P))
                    w_fc2.append(wt)
                fc2b_t = par.tile([P, ND], f32, tag="lnp")
                nc.sync.dma_start(fc2b_t[:], fc2_b[l].rearrange("(k p) -> p k", p=P))
                for do in range(ND):
                    acc = ps.tile([P, 256], f32, tag="acc")
                    for ft in range(NFT):
                        nc.tensor.matmul(
                            acc[:], lhsT=w_fc2[ft // 4][:, ft % 4, do * P:(do + 1) * P],
                            rhs=h1g[:, ft, :],
                            start=(ft == 0), stop=(ft == NFT - 1))
                    nc.vector.scalar_tensor_tensor(
                        out=x_sb[:, do, :], in0=acc[:], scalar=fc2b_t[:, do:do + 1],
                        in1=x_sb[:, do, :], op0=OP.add, op1=OP.add)

            # ---------------- final LN + AllGather + LM head ----------------
            lnfs_t = par.tile([P, ND], f32, tag="lnp")
            nc.sync.dma_start(lnfs_t[:], lnf_s.rearrange("(k p) -> p k", p=P))
            lnfb_t = par.tile([P, ND], f32, tag="lnp")
            nc.sync.dma_start(lnfb_t[:], lnf_b.rearrange("(k p) -> p k", p=P))
            xf_sb = act.tile([P, ND, 256], bf16, tag="h", bufs=2)
            _ln(nc, ps, act, rows, cons, x_sb, lnfs_t, lnfb_t, xf_sb)

            xf_in = dram.tile([ND, P, 256], bf16)
            xf_out = dram.tile([NCORES, ND, P, 256], bf16)
            nc.sync.dma_start(xf_in.rearrange("d p t -> p d t"), xf_sb[:])
            nc.gpsimd.collective_compute(
                "AllGather", OP.bypass, replica_groups=all_group,
                ins=[xf_in.opt()], outs=[xf_out.opt()])

            xall = []
            for g in range(4):
                xt = wpool.tile([P, 2, NT], bf16, tag="w", name=f"xall{g}")
                for i in range(2):
                    dt = 2 * g + i
                    nc.sync.dma_start(
                        xt[:, i, :].rearrange("p (r t) -> p r t", r=NCORES),
                        xf_out[:, dt, :, :].rearrange("r p t -> p r t"))
                xall.append(xt)

            lmb_t = par.tile([P, 32], f32, tag="fcb")
            nc.sync.dma_start(lmb_t[:], lm_b.rearrange("(k p) -> p k", p=P))
            for vt in range(VSP // P):
                lw = act.tile([P, ND, P], bf16, tag="lmw", bufs=3)
                nc.sync.dma_start(
                    lw[:], lm_wT[:, vt * P:(vt + 1) * P].rearrange(
                        "(d p) v -> p d v", p=P))
                for q4 in range(4):
                    acc = ps.tile([P, 512], f32, tag="acc")
                    for dt in range(ND):
                        nc.tensor.matmul(
                            acc[:], lhsT=lw[:, dt, :],
                            rhs=xall[dt // 2][:, dt % 2, q4 * 512:(q4 + 1) * 512],
                            start=(dt == 0), stop=(dt == ND - 1))
                    osb = act.tile([P, 512], f32, tag="osb", bufs=3)
                    nc.scalar.activation(osb[:], acc[:], AF.Identity,
                                         bias=lmb_t[:, vt:vt + 1])
                    nc.sync.dma_start(
                        outT[vt * P:(vt + 1) * P, q4 * 512:(q4 + 1) * 512], osb[:])

    nc.compile()
    return nc


def _prep_in_maps(inputs, n_layers=L):
    input_ids = np.asarray(inputs["input_ids"]).reshape(NT).astype(np.int32)
    pos_w = np.asarray(inputs["pos_w"], dtype=np.float32)
    embed_w = np.ascontiguousarray(np.asarray(inputs["embed_w"], dtype=np.float32))

    def t_bf(a, perm):
        return np.ascontiguousarray(
            np.transpose(np.asarray(a, dtype=np.float32), perm)
        ).astype(ml_dtypes.bfloat16)

    attn_wT = t_bf(inputs["attn_in_w"], (0, 2, 1))   # [L, D, 3D]
    proj_wT = t_bf(inputs["attn_out_w"], (0, 2, 1))  # [L, D(in), D(out)]
    fc1_wT = t_bf(inputs["fc1_w"], (0, 2, 1))        # [L, D, F]
    fc2_wT = t_bf(inputs["fc2_w"], (0, 2, 1))        # [L, F, D]

    lm_w = np.asarray(inputs["lm_w"], dtype=np.float32)
    lm_b_full = np.asarray(inputs["lm_b"], dtype=np.float32)

    common = {
        "embed_w": embed_w,
        "attn_wT": attn_wT,
        "attn_in_b": np.asarray(inputs["attn_in_b"], dtype=np.float32),
        "proj_wT": proj_wT,
        "proj_b": np.asarray(inputs["attn_out_b"], dtype=np.float32),
        "fc1_wT": fc1_wT,
        "fc1_b": np.asarray(inputs["fc1_b"], dtype=np.float32),
        "fc2_wT": fc2_wT,
        "fc2_b": np.asarray(inputs["fc2_b"], dtype=np.float32),
        "ln1_s": np.asarray(inputs["ln1_s"], dtype=np.float32),
        "ln1_b": np.asarray(inputs["ln1_b"], dtype=np.float32),
        "ln2_s": np.asarray(inputs["ln2_s"], dtype=np.float32),
        "ln2_b": np.asarray(inputs["ln2_b"], dtype=np.float32),
        "lnf_s": np.asarray(inputs["lnf_s"], dtype=np.float32),
        "lnf_b": np.asarray(inputs["lnf_b"], dtype=np.float32),
    }

    in_maps = []
    for c in range(NCORES):
        s0 = (c % 4) * T
        lm_shard = np.zeros((VSP, D), np.float32)
        lm_shard[:VS] = lm_w[c * VS:(c + 1) * VS]
        lmb_shard = np.zeros(VSP, np.float32)
        lmb_shard[:VS] = lm_b_full[c * VS:(c + 1) * VS]
        m = dict(common)
        m["ids"] = input_ids[c * T:(c + 1) * T]
        m["pos"] = np.ascontiguousarray(pos_w[s0:s0 + T])
        m["lm_wT"] = np.ascontiguousarray(lm_shard.T).astype(ml_dtypes.bfloat16)
        m["lm_b"] = lmb_shard
        in_maps.append(m)
    return in_maps


def _assemble(results):
    parts = []
    for c in range(NCORES):
        sh = results[c]["outT"][:VS, :]        # [4000, 2048]
        parts.append(sh.T)                     # [2048, 4000]
    logits = np.concatenate(parts, axis=1)     # [2048, 32000]
    return np.ascontiguousarray(logits.reshape(B, S, V).astype(np.float32))


_NC_CACHE = {}


def _get_nc(n_layers=L):
    if n_layers not in _NC_CACHE:
        _NC_CACHE[n_layers] = build(n_layers)
    return _NC_CACHE[n_layers]


def run(inputs, n_layers=L, trace=False):
    nc = _get_nc(n_layers)
    in_maps = _prep_in_maps(inputs, n_layers)
    res = bass_utils.run_bass_kernel_spmd(
        nc, in_maps, core_ids=list(range(NCORES)), trace=trace,
        trace_cores=[0] if trace else None)
    return _assemble(res.results), res


def kernel(**inputs) -> np.ndarray:
    out, _ = run(inputs)
    return out
